# revision 1
# baseline (speedup 1.0000x reference)
"""MoE layer (8 experts, top-4, + shared expert) on 8 Trainium2 NeuronCores.

Sharding: expert-parallel — core c owns expert c's FFN weights and a
1/8 column-slice of the shared expert; the router runs replicated on
every core. Each core produces a partial [N, H] output (its expert's
contribution weighted by the routing weight, plus its shared-expert
slice); the host sums the 8 partials.

SPMD trick: the program is identical on all cores, so core c's router
weight matrix is fed with its columns permuted so that expert c sits in
column 0. Top-k selection + normalization are permutation-invariant,
which makes "this core's routing weight" a fixed compile-time column.

Layout: all matmuls contract over the partition dim. Stage 1 computes
G^T/U^T = W^T X (features on partitions, tokens on free dim) so stage 2
(down-proj) can consume act^T directly as the stationary operand and
produce token-on-partition tiles — no transposes anywhere except the
tiny 8xN router logit transpose. Routing weights then apply as
per-partition scalars.

Router math: with scalar bias, top-4 of softmax scores == top-4 of
logits, and normalized top-k weights w_e = exp(l_e) * [l_e >= t4] /
sum_top4 exp(l_j) — the softmax denominator cancels, so no full softmax
is needed.

Dtype: float32r — a rounded fp32 variant that streams at bf16 rate on
the PE when the moving free dim is >= 256 (measured end-to-end rel err
2.5e-4 on hardware). Every producer feeding an FP32r matmul must itself
emit float32r, so the matmul-operand tiles and their DRAM sources are
declared float32r end-to-end (numpy arrays stay float32). bf16 was
measured only ~2% faster but with 500x worse absmax error (router
top-4 selection flips on ~2% of tokens); float16 NEFFs crash the exec
unit on this stack.

Performance model (per core): 313K PE rows at 1 cyc/row @ 2.4 GHz
~= 131 us floor; the schedule simulates at ~144 us with 92.6% PE
occupancy (DMA 26 MB fully overlapped except the ~4 us head). Device
loop measurements: ~160 us/iter in short bursts, ~179 us/iter
sustained — the delta is progressive PE power throttling under
sustained load, so a one-shot execution sits near the ~144 us model.
"""

import sys

if "/opt/trn_rl_repo" not in sys.path:
    sys.path.insert(0, "/opt/trn_rl_repo")

import numpy as np

B, S, H, E, I_DIM, IS = 2, 1024, 1024, 8, 512, 2048
N = B * S                 # 2048 tokens
NCORES = 8
ISS = IS // NCORES        # 256 shared-expert intermediate slice per core
P = 128                   # SBUF partitions
HC = H // P               # 8 contraction chunks over H
NB = 4                    # token blocks
TB = N // NB              # 512 tokens per block
NT = N // P               # 16 token tiles

import os as _os
MM_DTYPE = _os.environ.get("MOE_MM_DTYPE", "f32r")  # 'f32r'|'bf16'|'f32'

_CACHE = {}


def _build(mm_dtype, loop_reps=0, loop_hint=False):
    import concourse.mybir as mybir
    from concourse import bacc
    from concourse.masks import make_identity
    from concourse.tile import TileContext

    dt = mybir.dt
    f32 = dt.float32
    io_dt = {"bf16": dt.bfloat16, "f16": dt.float16, "f32r": dt.float32r, "f32": f32}[mm_dtype]

    nc = bacc.Bacc(None, target_bir_lowering=False, debug=False)

    xt_d = nc.declare_dram_parameter("xt", [H, N], io_dt, isOutput=False)
    rw_d = nc.declare_dram_parameter("rw", [P, HC * E], io_dt, isOutput=False)
    wg_d = nc.declare_dram_parameter("wg", [H, I_DIM], io_dt, isOutput=False)
    wu_d = nc.declare_dram_parameter("wu", [H, I_DIM], io_dt, isOutput=False)
    wd_d = nc.declare_dram_parameter("wd", [I_DIM, H], io_dt, isOutput=False)
    sg_d = nc.declare_dram_parameter("sg", [H, ISS], io_dt, isOutput=False)
    su_d = nc.declare_dram_parameter("su", [H, ISS], io_dt, isOutput=False)
    sd_d = nc.declare_dram_parameter("sd", [ISS, H], io_dt, isOutput=False)
    out_d = nc.declare_dram_parameter("out", [N, H], f32, isOutput=True)

    ACT = mybir.ActivationFunctionType
    ALU = mybir.AluOpType
    AXL = mybir.AxisListType

    def mm(out, lhsT, rhs, start, stop):
        nc.tensor.matmul(out, lhsT, rhs, start=start, stop=stop)

    with TileContext(nc) as tc:
        with (
            tc.tile_pool(name="persist", bufs=1) as pp,
            tc.tile_pool(name="tmp", bufs=3) as tpool,
            tc.tile_pool(name="ob", bufs=4) as opool,
            tc.tile_pool(name="ps", bufs=8, space="PSUM") as psp,
        ):

            def emit_body():
                # ---- persistent SBUF tiles + input DMAs ---------------
                ident8 = pp.tile([8, 8], f32, tag="ident8")
                make_identity(nc, ident8)

                # DMA issue order tracks the consumption order: router
                # weights + token block 0 first, then gate/up weights, then
                # the remaining token blocks interleaved with later weights.
                # rw rides the gpsimd (SWDGE) queue so it doesn't serialize
                # ahead of xt block 0 on the HWDGE queue.
                rw_sb = pp.tile([P, HC * E], io_dt, tag="rw")
                nc.gpsimd.dma_start(out=rw_sb, in_=rw_d[:, :])

                xt_sb = [[None] * NB for _ in range(HC)]
                wg_sb, wu_sb, sg_sb, su_sb = [], [], [], []
                wd_sb, sd_sb = [], []

                # block 0 fine-grained (per-chunk) to start the router ASAP
                for h in range(HC):
                    t = pp.tile([P, TB], io_dt, tag=f"xt{h}_0")
                    nc.sync.dma_start(out=t, in_=xt_d[h * P:(h + 1) * P, 0:TB])
                    xt_sb[h][0] = t

                def dma_xt_rest(b0, nblk):
                    # blocks b0..b0+nblk-1 merged per chunk: fewer DMAs to
                    # issue; consumption starts late enough that coarser
                    # arrival granularity costs nothing.
                    for h in range(HC):
                        t = pp.tile([P, nblk * TB], io_dt, tag=f"xt{h}_{b0}m")
                        nc.sync.dma_start(
                            out=t,
                            in_=xt_d[h * P:(h + 1) * P,
                                     b0 * TB:(b0 + nblk) * TB])
                        for j in range(nblk):
                            xt_sb[h][b0 + j] = t[:, j * TB:(j + 1) * TB]

                def dma_w(name, dram, width, lst, n):
                    for c in range(n):
                        t = pp.tile([P, width], io_dt, tag=f"{name}{c}")
                        nc.sync.dma_start(out=t,
                                          in_=dram[c * P:(c + 1) * P, :])
                        lst.append(t)

                for h in range(HC):
                    for name, dram, width, lst in (
                        ("wg", wg_d, I_DIM, wg_sb),
                        ("wu", wu_d, I_DIM, wu_sb),
                    ):
                        t = pp.tile([P, width], io_dt, tag=f"{name}{h}")
                        nc.sync.dma_start(out=t,
                                          in_=dram[h * P:(h + 1) * P, :])
                        lst.append(t)
                dma_xt_rest(1, 1)
                dma_w("sg", sg_d, ISS, sg_sb, HC)
                dma_w("su", su_d, ISS, su_sb, HC)
                dma_xt_rest(2, 2)
                dma_w("wd", wd_d, H, wd_sb, I_DIM // P)
                dma_w("sd", sd_d, H, sd_sb, ISS // P)

                # ---- router + gate/up stage 1, interleaved per token
                # block so program order matches DMA arrival order (each
                # engine executes its stream in-order, so emission order
                # is the schedule).
                actT = [[None] * NB for _ in range(I_DIM // P)]
                sactT = [[None] * NB for _ in range(ISS // P)]

                def stage1_pair(gW, uW, aT, it, nm, b):
                    isl = slice(it * P, (it + 1) * P)
                    pg = psp.tile([P, TB], f32, tag="ps")
                    for h in range(HC):
                        mm(pg, gW[h][:, isl], xt_sb[h][b],
                           start=(h == 0), stop=(h == HC - 1))
                    pu = psp.tile([P, TB], f32, tag="ps")
                    for h in range(HC):
                        mm(pu, uW[h][:, isl], xt_sb[h][b],
                           start=(h == 0), stop=(h == HC - 1))
                    # silu(g)*u as g*sigmoid(g)*u (CoreSim lacks Silu)
                    tmp = tpool.tile([P, TB], f32, tag="tmp")
                    nc.scalar.activation(tmp, pg, ACT.Sigmoid)
                    tmp2 = tpool.tile([P, TB], f32, tag="tmp")
                    nc.vector.tensor_tensor(out=tmp2, in0=tmp, in1=pu,
                                            op=ALU.mult)
                    at = pp.tile([P, TB], io_dt, tag=f"{nm}ct{it}_{b}")
                    nc.vector.tensor_tensor(out=at, in0=tmp2, in1=pg,
                                            op=ALU.mult)
                    aT[it][b] = at

                lps = psp.tile([P, NT * E], f32, tag="ps")
                for b in range(NB):
                    # router logits^T for block b -> [E, TB], then PE
                    # transpose to token-major L[p, t*8+e]
                    pr = psp.tile([E, TB], f32, tag="ps")
                    for h in range(HC):
                        mm(pr, rw_sb[:, h * E:(h + 1) * E], xt_sb[h][b],
                           start=(h == 0), stop=(h == HC - 1))
                    rt = tpool.tile([E, TB], f32, tag="tmp")
                    nc.vector.tensor_copy(rt, pr)
                    # first gate/up pair before the transposes: fills the
                    # PE wait on the DVE logit copy
                    stage1_pair(wg_sb, wu_sb, actT, 0, "a", b)
                    for tt in range(NT // NB):
                        t = b * (NT // NB) + tt
                        nc.tensor.transpose(
                            lps[:, t * E:(t + 1) * E],
                            rt[:, tt * P:(tt + 1) * P],
                            ident8,
                        )
                    for it in range(1, I_DIM // P):
                        stage1_pair(wg_sb, wu_sb, actT, it, "a", b)
                l_sb = pp.tile([P, NT * E], f32, tag="l_sb")
                nc.vector.tensor_copy(l_sb, lps)

                # exp(logits); top-4 threshold per token; masked weights
                e_sb = pp.tile([P, NT * E], f32, tag="e_sb")
                nc.scalar.activation(e_sb, l_sb, ACT.Exp)
                mx_sb = pp.tile([P, NT * E], f32, tag="mx_sb")
                for t in range(NT):
                    g = slice(t * E, (t + 1) * E)
                    nc.vector.max(out=mx_sb[:, g], in_=l_sb[:, g])
                msk_sb = pp.tile([P, NT * E], f32, tag="msk_sb")
                for t in range(NT):
                    g = slice(t * E, (t + 1) * E)
                    nc.vector.tensor_scalar(
                        out=msk_sb[:, g], in0=l_sb[:, g],
                        scalar1=mx_sb[:, t * E + 3:t * E + 4],
                        scalar2=None, op0=ALU.is_ge,
                    )
                w_sb = pp.tile([P, NT * E], f32, tag="w_sb")
                nc.vector.tensor_tensor(out=w_sb, in0=e_sb, in1=msk_sb,
                                        op=ALU.mult)
                d_sb = pp.tile([P, NT], f32, tag="d_sb")
                nc.vector.tensor_reduce(
                    out=d_sb, in_=w_sb.rearrange("p (t e) -> p t e", e=E),
                    axis=AXL.X, op=ALU.add,
                )
                r_sb = pp.tile([P, NT], f32, tag="r_sb")
                nc.vector.reciprocal(r_sb, d_sb)
                wfin = pp.tile([P, NT * E], f32, tag="wfin")
                for t in range(NT):
                    g = slice(t * E, (t + 1) * E)
                    nc.vector.tensor_scalar(
                        out=wfin[:, g], in0=w_sb[:, g],
                        scalar1=r_sb[:, t:t + 1], scalar2=None, op0=ALU.mult,
                    )

                # ---- shared-expert stage 1 (its weights stream in last) --
                for b in range(NB):
                    for it in range(ISS // P):
                        stage1_pair(sg_sb, su_sb, sactT, it, "s", b)

                # ---- stage 2: out = w0 * actT^T Wd + sactT^T sWd ------
                for t in range(NT):
                    b = t // (NT // NB)
                    o = (t % (NT // NB)) * P
                    wcol = wfin[:, t * E:t * E + 1]   # expert 0 == this core
                    for hb in range(2):
                        # finish both psum groups for this output half in 6
                        # matmuls so the scale/add/DMA chain starts early;
                        # 2 live psums per half also deepens the cross-tile
                        # pipeline in the 8-slot pool.
                        hsl = slice(hb * 512, (hb + 1) * 512)
                        pr = psp.tile([P, 512], f32, tag="ps")
                        for ic in range(I_DIM // P):
                            mm(pr, actT[ic][b][:, o:o + P], wd_sb[ic][:, hsl],
                               start=(ic == 0), stop=(ic == I_DIM // P - 1))
                        ps_ = psp.tile([P, 512], f32, tag="ps")
                        for sc in range(ISS // P):
                            mm(ps_, sactT[sc][b][:, o:o + P], sd_sb[sc][:, hsl],
                               start=(sc == 0), stop=(sc == ISS // P - 1))
                        # only one DVE input may live in PSUM: scale routed
                        # psum into SBUF, then add the shared psum.
                        ob = opool.tile([P, 512], f32, tag="ob")
                        nc.scalar.activation(ob, pr, ACT.Copy, scale=wcol)
                        nc.vector.tensor_tensor(out=ob, in0=ob, in1=ps_,
                                                op=ALU.add)
                        nc.sync.dma_start(
                            out=out_d[t * P:(t + 1) * P, hsl],
                            in_=ob,
                        )

            if loop_reps:
                hints = ()
                if loop_hint:
                    ET = mybir.EngineType
                    hints = (ET.PE, ET.DVE, ET.Activation, ET.SP, ET.Pool)
                with tc.For_i(0, loop_reps, 1, hint_engines=hints):
                    emit_body()
            else:
                emit_body()

    nc.compile()
    return nc


def _get_nc(mm_dtype=MM_DTYPE, loop_reps=0, loop_hint=False):
    key = (mm_dtype, loop_reps, loop_hint)
    if key not in _CACHE:
        _CACHE[key] = _build(mm_dtype, loop_reps, loop_hint)
    return _CACHE[key]


def make_in_maps(hidden_states, router_w, gate_w, up_w, down_w,
                 s_gate_w, s_up_w, s_down_w, mm_dtype=MM_DTYPE):
    if mm_dtype == "bf16":
        import ml_dtypes
        cvt = lambda a: np.ascontiguousarray(a).astype(ml_dtypes.bfloat16)
    elif mm_dtype == "f16":
        cvt = lambda a: np.ascontiguousarray(a).astype(np.float16)
    else:
        cvt = lambda a: np.ascontiguousarray(a, dtype=np.float32)

    xt = cvt(np.asarray(hidden_states).reshape(N, H).T)
    in_maps = []
    for c in range(NCORES):
        perm = [c] + [e for e in range(E) if e != c]
        # router weights packed to [P, HC*E]: row p holds chunks
        # (c, :) = rw[c*P + p, :] so the kernel slices per h-chunk.
        rw_packed = (np.asarray(router_w)[:, perm]
                     .reshape(HC, P, E).transpose(1, 0, 2).reshape(P, HC * E))
        in_maps.append({
            "xt": xt,
            "rw": cvt(rw_packed),
            "wg": cvt(np.asarray(gate_w)[c]),
            "wu": cvt(np.asarray(up_w)[c]),
            "wd": cvt(np.asarray(down_w)[c]),
            "sg": cvt(np.asarray(s_gate_w)[:, c * ISS:(c + 1) * ISS]),
            "su": cvt(np.asarray(s_up_w)[:, c * ISS:(c + 1) * ISS]),
            "sd": cvt(np.asarray(s_down_w)[c * ISS:(c + 1) * ISS, :]),
        })
    return in_maps


def kernel(hidden_states, router_w, router_bias, gate_w, up_w, down_w,
           s_gate_w, s_up_w, s_down_w):
    """Full-input MoE layer; returns [B, S, H] float32.

    router_bias is a scalar: it shifts all corrected scores equally, so
    it affects neither the top-k selection nor the weights — ignored.
    """
    import time

    from concourse.bass_utils import run_bass_kernel_spmd

    nc = _get_nc()
    in_maps = make_in_maps(hidden_states, router_w, gate_w, up_w, down_w,
                           s_gate_w, s_up_w, s_down_w)
    # the axon-tunneled device occasionally reports a transient
    # NRT_EXEC_UNIT_UNRECOVERABLE; a short pause + retry clears it.
    for attempt in range(3):
        try:
            res = run_bass_kernel_spmd(nc, in_maps, list(range(NCORES)))
            break
        except Exception:
            if attempt == 2:
                raise
            time.sleep(10)
    out = np.zeros((N, H), np.float32)
    for c in range(NCORES):
        out += res.results[c]["out"]
    return out.reshape(B, S, H)

